# revision 5
# baseline (speedup 1.0000x reference)
"""BiAttention (BiDAF-style) Trainium2 kernel, SPMD over 8 NeuronCores.

Reference computation (T = J = 8192, D = 100):
    S[i,j] = wc.c_i + wq.q_j + (wm*c_i).q_j
    A      = softmax_j(S)            # row softmax over question axis
    U_A    = A @ q                   # [T, D]  (C2Q)
    b      = max_j A                 # [T]
    h      = b @ c                   # [D]     (Q2C, global over T)
    G      = [c, U_A, c*U_A, c*h]    # [T, 4D]

Key algebraic facts used:
  * softmax rows are shift-invariant, so the wc.c_i term drops out entirely:
    A = softmax_j(q_j . (wq + wm*c_i)).
  * With W[k,i] = wq[k] + wm[k]*c[i,k]  (a [D, T] matrix, built on host),
    S~^T = q @ W, computed directly in [j-partition, i-free] layout so the
    second matmul (P^T contraction over j) needs no on-chip transposes.
  * Row sums Z come for free from an appended ones-column in q (row 100 of
    the U^T accumulator).  A = P/Z is never materialized; U_A = (P@q)/Z and
    b = max_j(P)/Z.

Sharding: context rows split 8 ways (1024 rows/core), full question per
core.  Softmax + C2Q fully local; Q2C needs one 100-float AllReduce of the
partial h = sum_i b_i*c_i.

Per-core device inputs:
    qa  [8192, 128] bf16 : q cast to bf16, col 100 = 1.0, cols 101:127 = 0
    w   [100, 1024] bf16 : W slice for this core's context rows
    c   [1024, 100] f32  : context slice
Output:
    out [1024, 400] f32  : G rows for this core
"""

import numpy as np
import ml_dtypes

T = 8192
J = 8192
D = 100
NCORES = 8
T_LOC = T // NCORES          # 1024 context rows per core
NB = T_LOC // 128            # 8 i-blocks of 128 rows
JT = J // 128                # 64 j-tiles of 128

BF16 = ml_dtypes.bfloat16

# Module-level knobs test.py may flip (kernel() defaults are what the
# grading harness uses).
TRACE = False
TRACE_KWARGS = {}
TMPDIR = None

_CACHE = {}


def _build_nc():
    import concourse.bacc as bacc
    import concourse.mybir as mybir
    import concourse.tile as tile

    nc = bacc.Bacc(None, target_bir_lowering=False, num_devices=NCORES)

    qa_d = nc.dram_tensor("qa", [J, 128], mybir.dt.bfloat16, kind="ExternalInput")
    w_d = nc.dram_tensor("w", [D, T_LOC], mybir.dt.bfloat16, kind="ExternalInput")
    c_d = nc.dram_tensor("c", [T_LOC, D], mybir.dt.float32, kind="ExternalInput")
    out_d = nc.dram_tensor("out", [T_LOC, 4 * D], mybir.dt.float32, kind="ExternalOutput")

    id_bf_d = nc.inline_tensor(np.eye(128, dtype=BF16), name="id_bf")
    id_f32_d = nc.inline_tensor(np.eye(128, dtype=np.float32), name="id_f32")

    FP32 = mybir.dt.float32
    BF = mybir.dt.bfloat16

    with tile.TileContext(nc) as tc:
        with (
            tc.tile_pool(name="const", bufs=1) as constp,
            tc.tile_pool(name="qa", bufs=4) as qap,
            tc.tile_pool(name="qt", bufs=4) as qtp,
            tc.tile_pool(name="pp", bufs=3) as ppool,
            tc.tile_pool(name="big", bufs=1) as bigp,
            tc.tile_pool(name="gg", bufs=NB) as gp,
            tc.tile_pool(name="small", bufs=3) as smallp,
            tc.tile_pool(name="ps_u", bufs=1, space="PSUM") as ps_u,
            tc.tile_pool(name="dram", bufs=1, space="DRAM") as dramp,
        ):
            # ---- constants / persistent state ----
            w_sb = constp.tile([128, T_LOC], BF, tag="w")
            nc.sync.dma_start(w_sb[0:D, :], w_d[:, :])
            idb_sb = constp.tile([128, 128], BF, tag="idb")
            nc.sync.dma_start(idb_sb[:], id_bf_d[:, :])
            idf_sb = constp.tile([128, 128], FP32, tag="idf")
            nc.sync.dma_start(idf_sb[:], id_f32_d[:, :])
            ones_sb = constp.tile([1, 128], FP32, tag="ones")
            nc.vector.memset(ones_sb[:], 1.0)

            c_sb = []
            for b in range(NB):
                cb = constp.tile([128, D], FP32, tag=f"c{b}")
                nc.sync.dma_start(cb[:], c_d[b * 128:(b + 1) * 128, :])
                c_sb.append(cb)

            # running elementwise max over j-tiles of P^T (bf16, [j-lane, i])
            macc = bigp.tile([128, T_LOC], BF, tag="macc")
            nc.vector.memset(macc[:], 0.0)

            # U^T accumulator: rows 0:100 = U^T = q^T @ P^T, row 100 = Z
            ut_ps = ps_u.tile([128, T_LOC], FP32, tag="ut")

            # ---- main loop over 64 j-tiles ----
            with tc.tile_pool(name="ps_s", bufs=2, space="PSUM") as ps_s:
                for t in range(JT):
                    qa_t = qap.tile([128, 128], BF, tag="qa")
                    nc.sync.dma_start(qa_t[:], qa_d[t * 128:(t + 1) * 128, :])
                    qt_t = qtp.tile([128, 128], BF, tag="qt")
                    nc.sync.dma_start_transpose(qt_t[:], qa_d[t * 128:(t + 1) * 128, :])

                    st = ps_s.tile([128, T_LOC], FP32, tag="st")
                    nc.tensor.matmul(st[:, 0:512], qt_t[0:D, :], w_sb[0:D, 0:512],
                                     start=True, stop=True)
                    nc.tensor.matmul(st[:, 512:1024], qt_t[0:D, :], w_sb[0:D, 512:1024],
                                     start=True, stop=True)

                    p_t = ppool.tile([128, T_LOC], BF, tag="p")
                    nc.scalar.activation(p_t[:], st[:], mybir.ActivationFunctionType.Exp)

                    nc.tensor.matmul(ut_ps[0:D + 1, 0:512], qa_t[:, 0:D + 1],
                                     p_t[:, 0:512],
                                     start=(t == 0), stop=(t == JT - 1))
                    nc.tensor.matmul(ut_ps[0:D + 1, 512:1024], qa_t[:, 0:D + 1],
                                     p_t[:, 512:1024],
                                     start=(t == 0), stop=(t == JT - 1))

                    nc.vector.tensor_max(macc[:], macc[:], p_t[:])

            # ---- tail: per-row stats, Q2C all-reduce, output assembly ----
            ut_sb = bigp.tile([128, T_LOC], FP32, tag="utsb")
            nc.vector.tensor_copy(ut_sb[0:D + 1, :], ut_ps[0:D + 1, :])

            with (
                tc.tile_pool(name="ps_t", bufs=2, space="PSUM") as ps_t,
                tc.tile_pool(name="ps_h", bufs=1, space="PSUM") as ps_h,
            ):
                h_ps = ps_h.tile([1, D], FP32, tag="h")
                g_tiles = []
                for b in range(NB):
                    sl = slice(b * 128, (b + 1) * 128)
                    # cross-partition max: transpose the max-acc block, then
                    # free-axis reduce
                    mtp = ps_t.tile([128, 128], BF, tag="mtpb")
                    nc.tensor.transpose(mtp[:], macc[:, sl], idb_sb[:])
                    maxc = smallp.tile([128, 1], FP32, tag="maxc")
                    nc.vector.reduce_max(maxc[:], mtp[:], axis=mybir.AxisListType.X)

                    # U block back to [i, d] layout; col 100 = Z
                    utp = ps_t.tile([128, 128], FP32, tag="tp")
                    nc.tensor.transpose(utp[:, 0:D + 1], ut_sb[0:D + 1, sl],
                                        idf_sb[0:D + 1, 0:D + 1])
                    rz = smallp.tile([128, 1], FP32, tag="rz")
                    nc.vector.reciprocal(rz[:], utp[:, D:D + 1])

                    g = gp.tile([128, 4 * D], FP32, tag="g")
                    nc.vector.tensor_scalar_mul(g[:, D:2 * D], utp[:, 0:D], rz[:])

                    bb = smallp.tile([128, 1], FP32, tag="bb")
                    nc.vector.tensor_mul(bb[:], maxc[:], rz[:])
                    nc.tensor.matmul(h_ps[:, :], bb[:], c_sb[b][:],
                                     start=(b == 0), stop=(b == NB - 1))

                    nc.vector.tensor_copy(g[:, 0:D], c_sb[b][:])
                    nc.vector.tensor_mul(g[:, 2 * D:3 * D], c_sb[b][:], g[:, D:2 * D])
                    g_tiles.append(g)

                # all-reduce the partial h across the 8 cores
                hp_sb = smallp.tile([1, D], FP32, tag="hp")
                nc.vector.tensor_copy(hp_sb[:], h_ps[:, :])
                cc_in = dramp.tile([1, D], FP32, tag="ccin")
                cc_out = dramp.tile([1, D], FP32, tag="ccout")
                nc.sync.dma_start(cc_in[:], hp_sb[:])
                nc.gpsimd.collective_compute(
                    "AllReduce",
                    mybir.AluOpType.add,
                    replica_groups=[list(range(NCORES))],
                    ins=[cc_in.opt()],
                    outs=[cc_out.opt()],
                )
                h_sb = smallp.tile([1, D], FP32, tag="hsb")
                nc.sync.dma_start(h_sb[:], cc_out[:])

                # broadcast h to 128 partitions via a K=1 outer-product matmul
                hb_ps = ps_t.tile([128, 128], FP32, tag="tp")
                nc.tensor.matmul(hb_ps[:, 0:D], ones_sb[:], h_sb[:],
                                 start=True, stop=True)
                hb_sb = constp.tile([128, D], FP32, tag="hbsb")
                nc.vector.tensor_copy(hb_sb[:], hb_ps[:, 0:D])

                for b in range(NB):
                    g = g_tiles[b]
                    nc.vector.tensor_mul(g[:, 3 * D:4 * D], c_sb[b][:], hb_sb[:])
                    nc.sync.dma_start(out_d[b * 128:(b + 1) * 128, :], g[:])

    nc.compile()
    return nc


def _get_nc():
    if "nc" not in _CACHE:
        _CACHE["nc"] = _build_nc()
    return _CACHE["nc"]


def kernel(context, question, kernel):
    from concourse.bass_utils import run_bass_kernel_spmd

    c = np.asarray(context, dtype=np.float32)[0]      # [T, D]
    q = np.asarray(question, dtype=np.float32)[0]     # [J, D]
    kv = np.asarray(kernel, dtype=np.float32)
    wq, wm = kv[D:2 * D], kv[2 * D:3 * D]             # wc drops out of softmax

    qa = np.zeros((J, 128), dtype=BF16)
    qa[:, :D] = q.astype(BF16)
    qa[:, D] = 1.0

    in_maps = []
    for m in range(NCORES):
        cm = c[m * T_LOC:(m + 1) * T_LOC]             # [T_LOC, D]
        W = (wq[:, None] + wm[:, None] * cm.T).astype(BF16)   # [D, T_LOC]
        in_maps.append({
            "qa": qa,
            "w": np.ascontiguousarray(W),
            "c": np.ascontiguousarray(cm),
        })

    nc = _get_nc()
    res = run_bass_kernel_spmd(
        nc, in_maps, core_ids=list(range(NCORES)),
        trace=TRACE, trace_kwargs=TRACE_KWARGS, tmpdir=TMPDIR,
    )
    _CACHE["last_results"] = res
    out = np.concatenate([res.results[m]["out"] for m in range(NCORES)], axis=0)
    return out.astype(np.float32)


# revision 6
# speedup vs baseline: 2.5254x; 2.5254x over previous
"""BiAttention (BiDAF-style) Trainium2 kernel, SPMD over 8 NeuronCores.

Reference computation (T = J = 8192, D = 100):
    S[i,j] = wc.c_i + wq.q_j + (wm*c_i).q_j
    A      = softmax_j(S)            # row softmax over question axis
    U_A    = A @ q                   # [T, D]  (C2Q)
    b      = max_j A                 # [T]
    h      = b @ c                   # [D]     (Q2C, global over T)
    G      = [c, U_A, c*U_A, c*h]    # [T, 4D]

Key algebraic facts used:
  * softmax rows are shift-invariant, so the wc.c_i term drops out entirely:
    A = softmax_j(q_j . (wq + wm*c_i)).
  * With W[k,i] = wq[k] + wm[k]*c[i,k]  (a [D, T] matrix, built on host),
    S~^T = q @ W, computed directly in [j-partition, i-free] layout so the
    second matmul (P^T contraction over j) needs no on-chip transposes.
  * Row sums Z come for free from an appended ones-column in q (row 100 of
    the U^T accumulator).  A = P/Z is never materialized; U_A = (P@q)/Z and
    b = max_j(P)/Z.

Sharding: context rows split 8 ways (1024 rows/core), full question per
core.  Softmax + C2Q fully local; Q2C needs one 100-float AllGather of the
partial h = sum_i b_i*c_i (summing + broadcasting the gathered partials is
a single K=8 matmul against a ones matrix).

Per-core device inputs:
    qa  [8192, 128] bf16    : q cast to bf16, col 100 = 1.0, rest 0
    qt  [64, 128, 128] bf16 : per-tile transposes of qa (q^T tiles)
    w   [100, 1024] bf16    : W slice for this core's context rows
    c   [1024, 100] f32     : context slice
Output:
    out [1024, 400] f32     : G rows for this core
"""

import numpy as np
import ml_dtypes

T = 8192
J = 8192
D = 100
NCORES = 8
T_LOC = T // NCORES          # 1024 context rows per core
NB = T_LOC // 128            # 8 i-blocks of 128 rows
JT = J // 128                # 64 j-tiles of 128

BF16 = ml_dtypes.bfloat16

# Module-level knobs test.py may flip (kernel() defaults are what the
# grading harness uses).
TRACE = False
TRACE_KWARGS = {}
TMPDIR = None

_CACHE = {}


def _build_nc():
    import concourse.bacc as bacc
    import concourse.mybir as mybir
    import concourse.tile as tile

    nc = bacc.Bacc(None, target_bir_lowering=False, num_devices=NCORES)

    qa_d = nc.dram_tensor("qa", [J, 128], mybir.dt.bfloat16, kind="ExternalInput")
    qt_d = nc.dram_tensor("qt", [JT, 128, 128], mybir.dt.bfloat16, kind="ExternalInput")
    w_d = nc.dram_tensor("w", [D, T_LOC], mybir.dt.bfloat16, kind="ExternalInput")
    c_d = nc.dram_tensor("c", [T_LOC, D], mybir.dt.float32, kind="ExternalInput")
    out_d = nc.dram_tensor("out", [T_LOC, 4 * D], mybir.dt.float32, kind="ExternalOutput")

    id_bf_d = nc.inline_tensor(np.eye(128, dtype=BF16), name="id_bf")
    id_f32_d = nc.inline_tensor(np.eye(128, dtype=np.float32), name="id_f32")

    FP32 = mybir.dt.float32
    BF = mybir.dt.bfloat16

    with tile.TileContext(nc) as tc:
        with (
            tc.tile_pool(name="const", bufs=1) as constp,
            tc.tile_pool(name="qa", bufs=JT) as qap,
            tc.tile_pool(name="qt", bufs=JT) as qtp,
            tc.tile_pool(name="pp", bufs=4) as ppool,
            tc.tile_pool(name="big", bufs=1) as bigp,
            tc.tile_pool(name="gg", bufs=NB) as gp,
            tc.tile_pool(name="small", bufs=3) as smallp,
            tc.tile_pool(name="ps_u", bufs=1, space="PSUM") as ps_u,
            tc.tile_pool(name="dram", bufs=1, space="DRAM") as dramp,
        ):
            # warm the ACT exp table immediately so the ~2.7us table load
            # overlaps the input DMAs instead of stalling the first real exp
            warm = constp.tile([1, 16], FP32, tag="warm")
            nc.vector.memset(warm[:], 0.0)
            nc.scalar.activation(warm[:], warm[:], mybir.ActivationFunctionType.Exp)

            # ---- constants / persistent state ----
            w_sb = constp.tile([128, T_LOC], BF, tag="w")
            nc.sync.dma_start(w_sb[0:D, :], w_d[:, :])
            idb_sb = constp.tile([128, 128], BF, tag="idb")
            nc.sync.dma_start(idb_sb[:], id_bf_d[:, :])
            idf_sb = constp.tile([128, 128], FP32, tag="idf")
            nc.sync.dma_start(idf_sb[:], id_f32_d[:, :])
            ones_sb = constp.tile([NCORES, 128], FP32, tag="ones")
            nc.vector.memset(ones_sb[:], 1.0)

            c_sb = []
            for b in range(NB):
                cb = constp.tile([128, D], FP32, tag=f"c{b}")
                nc.sync.dma_start(cb[:], c_d[b * 128:(b + 1) * 128, :])
                c_sb.append(cb)

            # all question tiles resident in SBUF (2 MB + 2 MB), prefetched
            # up-front on two different DMA paths
            qa_t = []
            qt_t = []
            for t in range(JT):
                at = qap.tile([128, 128], BF, tag="qa")
                nc.sync.dma_start(at[:], qa_d[t * 128:(t + 1) * 128, :])
                qa_t.append(at)
                tt = qtp.tile([128, 128], BF, tag="qt")
                nc.gpsimd.dma_start(tt[:], qt_d[t, :, :])
                qt_t.append(tt)

            # running elementwise max over j-tiles of P^T (bf16, [j-lane, i])
            macc = bigp.tile([128, T_LOC], BF, tag="macc")
            nc.vector.memset(macc[:], 0.0)

            # U^T accumulator: rows 0:100 = U^T = q^T @ P^T, row 100 = Z
            ut_ps = ps_u.tile([128, T_LOC], FP32, tag="ut")

            # ---- main loop over 64 j-tiles ----
            with tc.tile_pool(name="ps_s", bufs=2, space="PSUM") as ps_s:
                for t in range(JT):
                    st = ps_s.tile([128, T_LOC], FP32, tag="st")
                    nc.tensor.matmul(st[:, 0:512], qt_t[t][0:D, :], w_sb[0:D, 0:512],
                                     start=True, stop=True)
                    nc.tensor.matmul(st[:, 512:1024], qt_t[t][0:D, :],
                                     w_sb[0:D, 512:1024], start=True, stop=True)

                    p_t = ppool.tile([128, T_LOC], BF, tag="p")
                    nc.scalar.activation(p_t[:], st[:], mybir.ActivationFunctionType.Exp)

                    nc.tensor.matmul(ut_ps[0:D + 1, 0:512], qa_t[t][:, 0:D + 1],
                                     p_t[:, 0:512],
                                     start=(t == 0), stop=(t == JT - 1))
                    nc.tensor.matmul(ut_ps[0:D + 1, 512:1024], qa_t[t][:, 0:D + 1],
                                     p_t[:, 512:1024],
                                     start=(t == 0), stop=(t == JT - 1))

                    nc.vector.tensor_max(macc[:], macc[:], p_t[:])

            # ---- tail: per-row stats, Q2C all-gather, output assembly ----
            ut_sb = bigp.tile([128, T_LOC], FP32, tag="utsb")
            nc.vector.tensor_copy(ut_sb[0:D + 1, :], ut_ps[0:D + 1, :])

            with (
                tc.tile_pool(name="ps_t", bufs=2, space="PSUM") as ps_t,
                tc.tile_pool(name="ps_h", bufs=1, space="PSUM") as ps_h,
            ):
                h_ps = ps_h.tile([1, D], FP32, tag="h")
                g_tiles = []
                for b in range(NB):
                    sl = slice(b * 128, (b + 1) * 128)
                    # cross-partition max: transpose the max-acc block, then
                    # free-axis reduce
                    mtp = ps_t.tile([128, 128], BF, tag="mtpb")
                    nc.tensor.transpose(mtp[:], macc[:, sl], idb_sb[:])
                    maxc = smallp.tile([128, 1], FP32, tag="maxc")
                    nc.vector.reduce_max(maxc[:], mtp[:], axis=mybir.AxisListType.X)

                    # U block back to [i, d] layout; col 100 = Z
                    utp = ps_t.tile([128, 128], FP32, tag="tp")
                    nc.tensor.transpose(utp[:, 0:D + 1], ut_sb[0:D + 1, sl],
                                        idf_sb[0:D + 1, 0:D + 1])
                    rz = smallp.tile([128, 1], FP32, tag="rz")
                    nc.vector.reciprocal(rz[:], utp[:, D:D + 1])

                    g = gp.tile([128, 4 * D], FP32, tag="g")
                    nc.vector.tensor_scalar_mul(g[:, D:2 * D], utp[:, 0:D], rz[:])

                    bb = smallp.tile([128, 1], FP32, tag="bb")
                    nc.vector.tensor_mul(bb[:], maxc[:], rz[:])
                    nc.tensor.matmul(h_ps[:, :], bb[:], c_sb[b][:],
                                     start=(b == 0), stop=(b == NB - 1))

                    nc.vector.tensor_copy(g[:, 0:D], c_sb[b][:])
                    nc.vector.tensor_mul(g[:, 2 * D:3 * D], c_sb[b][:], g[:, D:2 * D])
                    g_tiles.append(g)

                # all-gather the partial h vectors (AG floor ~4.6us vs
                # AllReduce ~9.7us); sum+broadcast of the gathered [8, D]
                # is one K=8 matmul against ones
                hp_sb = smallp.tile([1, D], FP32, tag="hp")
                nc.vector.tensor_copy(hp_sb[:], h_ps[:, :])
                cc_in = dramp.tile([1, D], FP32, tag="ccin")
                cc_out = dramp.tile([NCORES, D], FP32, tag="ccout")
                nc.sync.dma_start(cc_in[:], hp_sb[:])
                nc.gpsimd.collective_compute(
                    "AllGather",
                    mybir.AluOpType.bypass,
                    replica_groups=[list(range(NCORES))],
                    ins=[cc_in.opt()],
                    outs=[cc_out.opt()],
                )
                gath_sb = smallp.tile([NCORES, D], FP32, tag="gath")
                nc.sync.dma_start(gath_sb[:], cc_out[:])

                # hb[p, d] = sum_r gath[r, d]  for every partition p
                hb_ps = ps_t.tile([128, 128], FP32, tag="tp")
                nc.tensor.matmul(hb_ps[:, 0:D], ones_sb[:], gath_sb[:],
                                 start=True, stop=True)
                hb_sb = constp.tile([128, D], FP32, tag="hbsb")
                nc.vector.tensor_copy(hb_sb[:], hb_ps[:, 0:D])

                for b in range(NB):
                    g = g_tiles[b]
                    nc.vector.tensor_mul(g[:, 3 * D:4 * D], c_sb[b][:], hb_sb[:])
                    nc.sync.dma_start(out_d[b * 128:(b + 1) * 128, :], g[:])

    nc.compile()
    return nc


def _get_nc():
    if "nc" not in _CACHE:
        _CACHE["nc"] = _build_nc()
    return _CACHE["nc"]


def kernel(context, question, kernel):
    from concourse.bass_utils import run_bass_kernel_spmd

    c = np.asarray(context, dtype=np.float32)[0]      # [T, D]
    q = np.asarray(question, dtype=np.float32)[0]     # [J, D]
    kv = np.asarray(kernel, dtype=np.float32)
    wq, wm = kv[D:2 * D], kv[2 * D:3 * D]             # wc drops out of softmax

    qa = np.zeros((J, 128), dtype=BF16)
    qa[:, :D] = q.astype(BF16)
    qa[:, D] = 1.0
    # per-tile transposes: qt[t] = qa[128t:128(t+1), :].T
    qt = np.ascontiguousarray(qa.reshape(JT, 128, 128).transpose(0, 2, 1))

    in_maps = []
    for m in range(NCORES):
        cm = c[m * T_LOC:(m + 1) * T_LOC]             # [T_LOC, D]
        W = (wq[:, None] + wm[:, None] * cm.T).astype(BF16)   # [D, T_LOC]
        in_maps.append({
            "qa": qa,
            "qt": qt,
            "w": np.ascontiguousarray(W),
            "c": np.ascontiguousarray(cm),
        })

    nc = _get_nc()
    res = run_bass_kernel_spmd(
        nc, in_maps, core_ids=list(range(NCORES)),
        trace=TRACE, trace_kwargs=TRACE_KWARGS, tmpdir=TMPDIR,
    )
    _CACHE["last_results"] = res
    out = np.concatenate([res.results[m]["out"] for m in range(NCORES)], axis=0)
    return out.astype(np.float32)
